# revision 4
# baseline (speedup 1.0000x reference)
# Self-contained Trainium2 Bass kernel for nn_MultiInputLSTMCell.
#
# Reference computation (all fp32):
#   pre   = h0 @ W_hh + bias + input_ @ W_ih          # (1, 3H)
#   i, o  = sigmoid(pre[:, :H]), sigmoid(pre[:, H:2H])
#   g     = tanh(pre[:, 2H:])
#   awi   = input_ @ aW_ih + a_bias                   # (1, H)
#   awh   = c_input @ aW_hh                           # (C, H)
#   alpha = sigmoid(awi + awh)                        # (C, H)
#   w     = exp([i; alpha]); w /= w.sum(0)            # (C+1, H)
#   c1    = (([g; c_input]) * w).sum(0)               # (1, H)
#   h1    = o * tanh(c1)
#
# Strategy: tensor-parallel over the hidden (output-column) dim across 8
# cores (HS = 256 columns each).  All elementwise/reduction work after the
# matmuls is local to a hidden shard, so no collectives are needed; the
# host scatters weight columns and gathers the (1, 256) h1/c1 shards.
#
# Per-core layout: the C axis lives on SBUF partitions, hidden on the free
# dim.  Weights are streamed through the PE as the *moving* operand (fp32
# moving operand streams 1 column/cycle); the tiny activation vectors are
# the stationary lhsT.  The (C+1)-axis softmax-style reduction is done with
# a K=65 ones-vector matmul.  The kernel is memory-bound on the ~16.8 MB of
# weights each core must read; DMAs are batched into >=1.5 MB transfers.

import numpy as np

import concourse.bass as bass
import concourse.tile as tile
from concourse import bacc, mybir
from concourse.bass_utils import run_bass_kernel_spmd

NCORES = 8
H = 2048          # hidden size
IN = 2048         # input size
C = 64            # number of skip-word cell states
HS = H // NCORES  # hidden shard per core = 256
KG = IN + H       # gates contraction dim = 4096
F32 = mybir.dt.float32

_nc_cache = None


def _build_nc():
    """Build the single-core Bass program (same program runs on all 8 cores)."""
    nc = bacc.Bacc(
        "TRN2",
        target_bir_lowering=False,
        debug=False,
        enable_asserts=False,
        name="multi_input_lstm_cell",
    )

    # DRAM I/O (per-core shards; shapes identical on every core)
    wg = nc.dram_tensor("wg", [KG, 3 * HS], F32, kind="ExternalInput").ap()
    wai = nc.dram_tensor("wai", [IN, HS], F32, kind="ExternalInput").ap()
    wah = nc.dram_tensor("wah", [H, HS], F32, kind="ExternalInput").ap()
    b = nc.dram_tensor("b", [1, 3 * HS], F32, kind="ExternalInput").ap()
    ab = nc.dram_tensor("ab", [1, HS], F32, kind="ExternalInput").ap()
    cs = nc.dram_tensor("cs", [C, HS], F32, kind="ExternalInput").ap()
    xt = nc.dram_tensor("xt", [128, KG // 128], F32, kind="ExternalInput").ap()
    ct = nc.dram_tensor("ct", [H, C], F32, kind="ExternalInput").ap()
    h1 = nc.dram_tensor("h1", [1, HS], F32, kind="ExternalOutput").ap()
    c1 = nc.dram_tensor("c1", [1, HS], F32, kind="ExternalOutput").ap()

    with tile.TileContext(nc) as tc:
        _emit(tc, wg, wai, wah, b, ab, cs, xt, ct, h1, c1)

    nc.compile()
    return nc


def _emit(tc, wg, wai, wah, b, ab, cs, xt, ct, h1, c1):
    from contextlib import ExitStack

    nc = tc.nc
    KO_G = KG // 128          # 32 contraction chunks for the gates matmul
    KO_A = IN // 128          # 16 contraction chunks for the alpha matmuls
    GSUB = 4                  # gates k-chunks per DMA (tile = [128, 4, 768] = 1.5 MB)

    with ExitStack() as ctx:
        singles = ctx.enter_context(tc.tile_pool(name="singles", bufs=1))
        wg_pool = ctx.enter_context(tc.tile_pool(name="wg_pool", bufs=3))
        psum = ctx.enter_context(tc.tile_pool(name="psum", bufs=1, space="PSUM"))

        # ---- small resident tiles -------------------------------------
        xt_t = singles.tile([128, KO_G], F32, tag="xt")
        nc.sync.dma_start(out=xt_t[:], in_=xt)

        ct_t = singles.tile([128, KO_A, C], F32, tag="ct")
        nc.sync.dma_start(out=ct_t[:], in_=ct.rearrange("(ko ki) c -> ki ko c", ki=128))

        b_t = singles.tile([1, 3 * HS], F32, tag="b")
        nc.sync.dma_start(out=b_t[:], in_=b)
        ab_t = singles.tile([1, HS], F32, tag="ab")
        nc.sync.dma_start(out=ab_t[:], in_=ab)

        # merge tile rows: [c_input-shard; g]  (C+1 = 65 partitions; the
        # singleton gate row lives at partition 64 — compute instructions
        # only support start partitions {0, 32, 64})
        mg_t = singles.tile([C + 1, HS], F32, tag="mg")
        nc.sync.dma_start(out=mg_t[0:C, :], in_=cs)

        ew_t = singles.tile([C + 1, HS], F32, tag="ew")

        ones_r = singles.tile([C + 1, 1], F32, tag="ones_r")   # reduction lhsT
        nc.vector.memset(ones_r[:], 1.0)
        ones_b = singles.tile([1, C], F32, tag="ones_b")       # broadcast lhsT
        nc.vector.memset(ones_b[:], 1.0)

        # ---- whole-tensor alpha weight loads (2 MB each) --------------
        wai_t = singles.tile([128, KO_A, HS], F32, tag="wai")
        nc.sync.dma_start(out=wai_t[:], in_=wai.rearrange("(ko ki) n -> ki ko n", ki=128))
        wah_t = singles.tile([128, KO_A, HS], F32, tag="wah")
        nc.sync.dma_start(out=wah_t[:], in_=wah.rearrange("(ko ki) n -> ki ko n", ki=128))

        # ---- PSUM tiles ----------------------------------------------
        pg_a = psum.tile([1, 512], F32, tag="pg_a")    # gates cols 0..512 (i, o)
        pg_b = psum.tile([1, HS], F32, tag="pg_b")     # gates cols 512..768 (g)
        pwi = psum.tile([1, HS], F32, tag="pwi")       # alpha_wi row
        pal = psum.tile([C, HS], F32, tag="pal")       # alpha pre-activation
        ps0 = psum.tile([1, HS], F32, tag="ps0")       # sum(exp(logits))
        ps1 = psum.tile([1, HS], F32, tag="ps1")       # sum(merge * exp(logits))

        # ---- alpha_wi = input_ @ aW_ih  (input_ = xt cols 16..31) -----
        for ko in range(KO_A):
            nc.tensor.matmul(
                pwi[:],
                lhsT=xt_t[:, KO_A + ko : KO_A + ko + 1],
                rhs=wai_t[:, ko, :],
                start=(ko == 0),
                stop=(ko == KO_A - 1),
            )

        # ---- alpha pre = c_input @ aW_hh  (+ broadcast wi row later) --
        for ko in range(KO_A):
            nc.tensor.matmul(
                pal[:],
                lhsT=ct_t[:, ko, :],
                rhs=wah_t[:, ko, :],
                start=(ko == 0),
                stop=False,
            )

        # wi row (+ alpha_bias) to SBUF, then broadcast-add into pal via a
        # K=1 rank-1 matmul with a ones column.
        wi_t = singles.tile([1, HS], F32, tag="wi")
        nc.vector.tensor_add(out=wi_t[:], in0=pwi[:], in1=ab_t[:])
        nc.tensor.matmul(
            pal[:], lhsT=ones_b[:], rhs=wi_t[:], start=False, stop=True,
        )

        # ---- gates: [h0|input_] @ [W_hh; W_ih] shard ------------------
        wg_r = wg.rearrange("(ko km ki) n -> ko ki km n", ki=128, km=GSUB)
        for ko in range(KO_G // GSUB):
            wg_t = wg_pool.tile([128, GSUB, 3 * HS], F32, tag="wg")
            nc.sync.dma_start(out=wg_t[:], in_=wg_r[ko])
            for km in range(GSUB):
                kk = ko * GSUB + km
                nc.tensor.matmul(
                    pg_a[:],
                    lhsT=xt_t[:, kk : kk + 1],
                    rhs=wg_t[:, km, 0:512],
                    start=(kk == 0),
                    stop=(kk == KO_G - 1),
                )
                nc.tensor.matmul(
                    pg_b[:],
                    lhsT=xt_t[:, kk : kk + 1],
                    rhs=wg_t[:, km, 512 : 3 * HS],
                    start=(kk == 0),
                    stop=(kk == KO_G - 1),
                )

        # ---- gate activations ----------------------------------------
        ga_t = singles.tile([1, 512], F32, tag="ga")
        nc.vector.tensor_add(out=ga_t[:], in0=pg_a[:], in1=b_t[:, 0:512])
        gb_t = singles.tile([1, HS], F32, tag="gb")
        nc.vector.tensor_add(out=gb_t[:], in0=pg_b[:], in1=b_t[:, 512 : 3 * HS])

        og_t = singles.tile([1, HS], F32, tag="og")
        # ew row 64 = i gate; mg row 64 = g candidate
        nc.scalar.activation(
            out=ew_t[C : C + 1, :], in_=ga_t[:, 0:HS],
            func=mybir.ActivationFunctionType.Sigmoid,
        )
        nc.scalar.activation(
            out=og_t[:], in_=ga_t[:, HS:512],
            func=mybir.ActivationFunctionType.Sigmoid,
        )
        nc.scalar.activation(
            out=mg_t[C : C + 1, :], in_=gb_t[:],
            func=mybir.ActivationFunctionType.Tanh,
        )
        # ew rows 0..63 = alpha = sigmoid(pal)
        nc.scalar.activation(
            out=ew_t[0:C, :], in_=pal[:],
            func=mybir.ActivationFunctionType.Sigmoid,
        )
        # exp of the [i; alpha] logits (rows 0..64)
        nc.scalar.activation(
            out=ew_t[:], in_=ew_t[:],
            func=mybir.ActivationFunctionType.Exp,
        )
        # merge * exp(logits)
        nc.vector.tensor_mul(out=mg_t[:], in0=mg_t[:], in1=ew_t[:])

        # ---- reduce over the (C+1) axis via ones-matmul (K = 65) ------
        nc.tensor.matmul(ps0[:], lhsT=ones_r[:], rhs=ew_t[:], start=True, stop=True)
        nc.tensor.matmul(ps1[:], lhsT=ones_r[:], rhs=mg_t[:], start=True, stop=True)

        # ---- c1 = ps1 / ps0 ; h1 = o * tanh(c1) -----------------------
        r_t = singles.tile([1, HS], F32, tag="r")
        nc.vector.reciprocal(out=r_t[:], in_=ps0[:])
        c1_t = singles.tile([1, HS], F32, tag="c1")
        nc.vector.tensor_mul(out=c1_t[:], in0=ps1[:], in1=r_t[:])

        t_t = singles.tile([1, HS], F32, tag="t")
        nc.scalar.activation(
            out=t_t[:], in_=c1_t[:], func=mybir.ActivationFunctionType.Tanh
        )
        h1_t = singles.tile([1, HS], F32, tag="h1")
        nc.vector.tensor_mul(out=h1_t[:], in0=og_t[:], in1=t_t[:])

        nc.sync.dma_start(out=c1, in_=c1_t[:])
        nc.sync.dma_start(out=h1, in_=h1_t[:])


def _shard_inputs(input_, c_input, h0, c0, weight_ih, weight_hh,
                  alpha_weight_ih, alpha_weight_hh, bias, alpha_bias):
    """Host-side scatter: column-shard the weights over the hidden dim."""
    f32 = np.float32
    x_comb = np.concatenate([h0[0], input_[0]]).astype(f32)          # (4096,)
    xt = np.ascontiguousarray(x_comb.reshape(KG // 128, 128).T)      # (128, 32)
    ct = np.ascontiguousarray(c_input.T.astype(f32))                 # (2048, 64)

    in_maps = []
    for k in range(NCORES):
        cols = np.s_[k * HS : (k + 1) * HS]
        gcols = np.r_[0 * H + k * HS : 0 * H + (k + 1) * HS,
                      1 * H + k * HS : 1 * H + (k + 1) * HS,
                      2 * H + k * HS : 2 * H + (k + 1) * HS]
        wg = np.ascontiguousarray(
            np.concatenate([weight_hh[:, gcols], weight_ih[:, gcols]], axis=0)
        ).astype(f32)                                                # (4096, 768)
        in_maps.append({
            "wg": wg,
            "wai": np.ascontiguousarray(alpha_weight_ih[:, cols]).astype(f32),
            "wah": np.ascontiguousarray(alpha_weight_hh[:, cols]).astype(f32),
            "b": np.ascontiguousarray(bias[gcols])[None, :].astype(f32),
            "ab": np.ascontiguousarray(alpha_bias[cols])[None, :].astype(f32),
            "cs": np.ascontiguousarray(c_input[:, cols]).astype(f32),
            "xt": xt,
            "ct": ct,
        })
    return in_maps


def _run(inputs, trace=False):
    global _nc_cache
    if _nc_cache is None:
        _nc_cache = _build_nc()
    nc = _nc_cache
    in_maps = _shard_inputs(**inputs)
    res = run_bass_kernel_spmd(nc, in_maps, core_ids=list(range(NCORES)), trace=trace)
    h1 = np.concatenate([res.results[k]["h1"] for k in range(NCORES)], axis=1)
    c1 = np.concatenate([res.results[k]["c1"] for k in range(NCORES)], axis=1)
    return (h1.astype(np.float32), c1.astype(np.float32)), res


def kernel(input_, c_input, h0, c0, weight_ih, weight_hh,
           alpha_weight_ih, alpha_weight_hh, bias, alpha_bias):
    inputs = dict(
        input_=np.asarray(input_, np.float32),
        c_input=np.asarray(c_input, np.float32),
        h0=np.asarray(h0, np.float32),
        c0=np.asarray(c0, np.float32),
        weight_ih=np.asarray(weight_ih, np.float32),
        weight_hh=np.asarray(weight_hh, np.float32),
        alpha_weight_ih=np.asarray(alpha_weight_ih, np.float32),
        alpha_weight_hh=np.asarray(alpha_weight_hh, np.float32),
        bias=np.asarray(bias, np.float32),
        alpha_bias=np.asarray(alpha_bias, np.float32),
    )
    out, _ = _run(inputs)
    return out


# revision 7
# speedup vs baseline: 1.0228x; 1.0228x over previous
# Self-contained Trainium2 Bass kernel for nn_MultiInputLSTMCell.
#
# Reference computation (all fp32):
#   pre   = h0 @ W_hh + bias + input_ @ W_ih          # (1, 3H)
#   i, o  = sigmoid(pre[:, :H]), sigmoid(pre[:, H:2H])
#   g     = tanh(pre[:, 2H:])
#   awi   = input_ @ aW_ih + a_bias                   # (1, H)
#   awh   = c_input @ aW_hh                           # (C, H)
#   alpha = sigmoid(awi + awh)                        # (C, H)
#   w     = exp([i; alpha]); w /= w.sum(0)            # (C+1, H)
#   c1    = (([g; c_input]) * w).sum(0)               # (1, H)
#   h1    = o * tanh(c1)
#
# Strategy: tensor-parallel over the hidden (output-column) dim across 8
# cores (HS = 256 columns each).  All elementwise/reduction work after the
# matmuls is local to a hidden shard, so no collectives are needed; the
# host scatters weight columns and gathers the (1, 256) h1/c1 shards.
#
# Per-core layout: the C axis lives on SBUF partitions, hidden on the free
# dim.  Weights are streamed through the PE as the *moving* operand in
# float32r (single-pass fp32: 1 col/cycle vs 4 for exact fp32); the tiny
# activation vectors are the stationary lhsT.  The (C+1)-axis softmax-style
# reduction is a K=65 ones-vector matmul kept in exact fp32.  The kernel is
# memory-bound on the ~16.6 MB of weights each core reads; weight DMAs are
# batched into 2-3 MB transfers on the sync HWDGE ring while small operands
# ride the scalar ring.

import numpy as np

import concourse.bass as bass
import concourse.tile as tile
from concourse import bacc, mybir
from concourse.bass_utils import run_bass_kernel_spmd

NCORES = 8
H = 2048          # hidden size
IN = 2048         # input size
C = 64            # number of skip-word cell states
HS = H // NCORES  # hidden shard per core = 256
KG = IN + H       # gates contraction dim = 4096
F32 = mybir.dt.float32
F32R = mybir.dt.float32r

_nc_cache = None


def _build_nc():
    """Build the single-core Bass program (same program runs on all 8 cores)."""
    nc = bacc.Bacc(
        "TRN2",
        target_bir_lowering=False,
        debug=False,
        enable_asserts=False,
        name="multi_input_lstm_cell",
    )

    # DRAM I/O (per-core shards; shapes identical on every core)
    wg = nc.dram_tensor("wg", [KG, 3 * HS], F32R, kind="ExternalInput").ap()
    wai = nc.dram_tensor("wai", [IN, HS], F32R, kind="ExternalInput").ap()
    wah = nc.dram_tensor("wah", [H, HS], F32R, kind="ExternalInput").ap()
    b = nc.dram_tensor("b", [1, 3 * HS], F32, kind="ExternalInput").ap()
    ab = nc.dram_tensor("ab", [1, HS], F32, kind="ExternalInput").ap()
    cs = nc.dram_tensor("cs", [C, HS], F32, kind="ExternalInput").ap()
    xt = nc.dram_tensor("xt", [128, KG // 128], F32R, kind="ExternalInput").ap()
    ct = nc.dram_tensor("ct", [H, C], F32R, kind="ExternalInput").ap()
    # hc[0, 0:256] = c1 shard, hc[0, 256:512] = h1 shard (one output DMA)
    hc = nc.dram_tensor("hc", [1, 2 * HS], F32, kind="ExternalOutput").ap()

    with tile.TileContext(nc) as tc:
        _emit(tc, wg, wai, wah, b, ab, cs, xt, ct, hc)

    nc.compile()
    return nc


def _emit(tc, wg, wai, wah, b, ab, cs, xt, ct, hc):
    from contextlib import ExitStack

    nc = tc.nc
    KO_G = KG // 128          # 32 contraction chunks for the gates matmul
    KO_A = IN // 128          # 16 contraction chunks for the alpha matmuls
    GSUB = 8                  # gates k-chunks per DMA (tile = [128, 8, 768] = 3 MB)
    SIG = mybir.ActivationFunctionType.Sigmoid
    TANH = mybir.ActivationFunctionType.Tanh
    EXP = mybir.ActivationFunctionType.Exp

    with ExitStack() as ctx:
        singles = ctx.enter_context(tc.tile_pool(name="singles", bufs=1))
        wg_pool = ctx.enter_context(tc.tile_pool(name="wg_pool", bufs=3))
        psum = ctx.enter_context(tc.tile_pool(name="psum", bufs=1, space="PSUM"))

        # ---- big weight streams on the sync (SP) HWDGE ring -----------
        wai_t = singles.tile([128, KO_A, HS], F32R, tag="wai")
        nc.sync.dma_start(out=wai_t[:], in_=wai.rearrange("(ko ki) n -> ki ko n", ki=128))
        wah_t = singles.tile([128, KO_A, HS], F32R, tag="wah")
        nc.sync.dma_start(out=wah_t[:], in_=wah.rearrange("(ko ki) n -> ki ko n", ki=128))

        # ---- small resident tiles on the scalar (ACT) HWDGE ring ------
        xt_t = singles.tile([128, KO_G], F32R, tag="xt")
        nc.scalar.dma_start(out=xt_t[:], in_=xt)

        ct_t = singles.tile([128, KO_A, C], F32R, tag="ct")
        nc.scalar.dma_start(out=ct_t[:], in_=ct.rearrange("(ko ki) c -> ki ko c", ki=128))

        b_t = singles.tile([1, 3 * HS], F32, tag="b")
        nc.scalar.dma_start(out=b_t[:], in_=b)
        ab_t = singles.tile([1, HS], F32, tag="ab")
        nc.scalar.dma_start(out=ab_t[:], in_=ab)

        # merge tile rows: [c_input-shard; g]  (C+1 = 65 partitions; the
        # singleton gate row lives at partition 64 — compute instructions
        # only support start partitions {0, 32, 64})
        mg_t = singles.tile([C + 1, HS], F32, tag="mg")
        nc.scalar.dma_start(out=mg_t[0:C, :], in_=cs)

        ew_t = singles.tile([C + 1, HS], F32, tag="ew")

        ones_r = singles.tile([C + 1, 1], F32, tag="ones_r")   # reduction lhsT
        nc.vector.memset(ones_r[:], 1.0)
        ones_b = singles.tile([1, C], F32, tag="ones_b")       # broadcast/bias lhsT
        nc.vector.memset(ones_b[:], 1.0)

        # Pre-warm the ACT engine's exp table (slot 1) while everything is
        # idle so the mid-kernel exp doesn't pay the ~1.3 µs table load.
        warm_t = singles.tile([1, 1], F32, tag="warm")
        nc.vector.memset(warm_t[:], 0.0)
        nc.scalar.activation(out=warm_t[:], in_=warm_t[:], func=EXP)

        # ---- PSUM tiles ----------------------------------------------
        pg_a = psum.tile([1, 512], F32, tag="pg_a")    # gates cols 0..512 (i, o)
        pg_b = psum.tile([1, HS], F32, tag="pg_b")     # gates cols 512..768 (g)
        pwi = psum.tile([1, HS], F32, tag="pwi")       # alpha_wi row
        pal = psum.tile([C, HS], F32, tag="pal")       # alpha pre-activation
        ps0 = psum.tile([1, HS], F32, tag="ps0")       # sum(exp(logits))
        ps1 = psum.tile([1, HS], F32, tag="ps1")       # sum(merge * exp(logits))

        # ---- alpha_wi = input_ @ aW_ih  (input_ = xt cols 16..31) -----
        for ko in range(KO_A):
            nc.tensor.matmul(
                pwi[:],
                lhsT=xt_t[:, KO_A + ko : KO_A + ko + 1],
                rhs=wai_t[:, ko, :],
                start=(ko == 0),
                stop=(ko == KO_A - 1),
            )

        # ---- alpha pre = c_input @ aW_hh  (+ broadcast wi row later) --
        for ko in range(KO_A):
            nc.tensor.matmul(
                pal[:],
                lhsT=ct_t[:, ko, :],
                rhs=wah_t[:, ko, :],
                start=(ko == 0),
                stop=False,
            )

        # wi row (+ alpha_bias) to SBUF, then broadcast-add into pal via a
        # K=1 rank-1 matmul with a ones column.
        wi_t = singles.tile([1, HS], F32, tag="wi")
        nc.vector.tensor_add(out=wi_t[:], in0=pwi[:], in1=ab_t[:])
        nc.tensor.matmul(
            pal[:], lhsT=ones_b[0:1, 0:C], rhs=wi_t[:], start=False, stop=True,
        )
        # alpha rows of the exp-normalize input: ew[0:64] = sigmoid(pal)
        nc.scalar.activation(out=ew_t[0:C, :], in_=pal[:], func=SIG)

        # ---- gates: bias + [h0|input_] @ [W_hh; W_ih] shard -----------
        # The bias row is folded into the PSUM accumulation as a K=1 matmul
        # so no separate elementwise add sits on the critical-path tail.
        nc.tensor.matmul(pg_a[:], lhsT=ones_b[0:1, 0:1], rhs=b_t[:, 0:512],
                         start=True, stop=False)
        nc.tensor.matmul(pg_b[:], lhsT=ones_b[0:1, 0:1], rhs=b_t[:, 512 : 3 * HS],
                         start=True, stop=False)
        wg_r = wg.rearrange("(ko km ki) n -> ko ki km n", ki=128, km=GSUB)
        for ko in range(KO_G // GSUB):
            wg_t = wg_pool.tile([128, GSUB, 3 * HS], F32R, tag="wg")
            nc.sync.dma_start(out=wg_t[:], in_=wg_r[ko])
            for km in range(GSUB):
                kk = ko * GSUB + km
                nc.tensor.matmul(
                    pg_a[:],
                    lhsT=xt_t[:, kk : kk + 1],
                    rhs=wg_t[:, km, 0:512],
                    start=False,
                    stop=(kk == KO_G - 1),
                )
                nc.tensor.matmul(
                    pg_b[:],
                    lhsT=xt_t[:, kk : kk + 1],
                    rhs=wg_t[:, km, 512 : 3 * HS],
                    start=False,
                    stop=(kk == KO_G - 1),
                )

        # ---- tail: activations, exp-normalize, h1/c1 ------------------
        # ew row 64 = i gate; mg row 64 = g candidate
        nc.scalar.activation(out=ew_t[C : C + 1, :], in_=pg_a[:, 0:HS], func=SIG)
        nc.scalar.activation(out=mg_t[C : C + 1, :], in_=pg_b[:], func=TANH)
        # exp of the [alpha; i] logits (rows 0..64)
        nc.scalar.activation(out=ew_t[:], in_=ew_t[:], func=EXP)
        # o gate is only needed for the final product — keep it off the
        # critical path (after exp on the ACT queue).
        og_t = singles.tile([1, HS], F32, tag="og")
        nc.scalar.activation(out=og_t[:], in_=pg_a[:, HS:512], func=SIG)
        # merge * exp(logits)
        nc.vector.tensor_mul(out=mg_t[:], in0=mg_t[:], in1=ew_t[:])

        # ---- reduce over the (C+1) axis via ones-matmul (K = 65) ------
        nc.tensor.matmul(ps0[:], lhsT=ones_r[:], rhs=ew_t[:], start=True, stop=True)
        nc.tensor.matmul(ps1[:], lhsT=ones_r[:], rhs=mg_t[:], start=True, stop=True)

        # ---- c1 = ps1 / ps0 ; h1 = o * tanh(c1) -----------------------
        # s0 = sum of 65 exp values in [1, e] — safely normal, so the
        # fast reciprocal approximation (~18 good bits) is plenty.
        r_t = singles.tile([1, HS], F32, tag="r")
        nc.vector.reciprocal_approx_fast(out=r_t[:], in_=ps0[:])
        hc_t = singles.tile([1, 2 * HS], F32, tag="hc")
        c1_t = hc_t[:, 0:HS]
        nc.vector.tensor_mul(out=c1_t, in0=ps1[:], in1=r_t[:])

        t_t = singles.tile([1, HS], F32, tag="t")
        nc.scalar.activation(out=t_t[:], in_=c1_t, func=TANH)
        nc.vector.tensor_mul(out=hc_t[:, HS : 2 * HS], in0=og_t[:], in1=t_t[:])

        nc.sync.dma_start(out=hc, in_=hc_t[:])


def _shard_inputs(input_, c_input, h0, c0, weight_ih, weight_hh,
                  alpha_weight_ih, alpha_weight_hh, bias, alpha_bias):
    """Host-side scatter: column-shard the weights over the hidden dim."""
    f32 = np.float32
    x_comb = np.concatenate([h0[0], input_[0]]).astype(f32)          # (4096,)
    xt = np.ascontiguousarray(x_comb.reshape(KG // 128, 128).T)      # (128, 32)
    ct = np.ascontiguousarray(c_input.T.astype(f32))                 # (2048, 64)

    in_maps = []
    for k in range(NCORES):
        cols = np.s_[k * HS : (k + 1) * HS]
        gcols = np.r_[0 * H + k * HS : 0 * H + (k + 1) * HS,
                      1 * H + k * HS : 1 * H + (k + 1) * HS,
                      2 * H + k * HS : 2 * H + (k + 1) * HS]
        wg = np.ascontiguousarray(
            np.concatenate([weight_hh[:, gcols], weight_ih[:, gcols]], axis=0)
        ).astype(f32)                                                # (4096, 768)
        in_maps.append({
            "wg": wg,
            "wai": np.ascontiguousarray(alpha_weight_ih[:, cols]).astype(f32),
            "wah": np.ascontiguousarray(alpha_weight_hh[:, cols]).astype(f32),
            "b": np.ascontiguousarray(bias[gcols])[None, :].astype(f32),
            "ab": np.ascontiguousarray(alpha_bias[cols])[None, :].astype(f32),
            "cs": np.ascontiguousarray(c_input[:, cols]).astype(f32),
            "xt": xt,
            "ct": ct,
        })
    return in_maps


def _run(inputs, trace=False):
    global _nc_cache
    if _nc_cache is None:
        _nc_cache = _build_nc()
    nc = _nc_cache
    in_maps = _shard_inputs(**inputs)
    res = run_bass_kernel_spmd(nc, in_maps, core_ids=list(range(NCORES)), trace=trace)
    h1 = np.concatenate(
        [res.results[k]["hc"][:, HS : 2 * HS] for k in range(NCORES)], axis=1)
    c1 = np.concatenate(
        [res.results[k]["hc"][:, 0:HS] for k in range(NCORES)], axis=1)
    return (h1.astype(np.float32), c1.astype(np.float32)), res


def kernel(input_, c_input, h0, c0, weight_ih, weight_hh,
           alpha_weight_ih, alpha_weight_hh, bias, alpha_bias):
    inputs = dict(
        input_=np.asarray(input_, np.float32),
        c_input=np.asarray(c_input, np.float32),
        h0=np.asarray(h0, np.float32),
        c0=np.asarray(c0, np.float32),
        weight_ih=np.asarray(weight_ih, np.float32),
        weight_hh=np.asarray(weight_hh, np.float32),
        alpha_weight_ih=np.asarray(alpha_weight_ih, np.float32),
        alpha_weight_hh=np.asarray(alpha_weight_hh, np.float32),
        bias=np.asarray(bias, np.float32),
        alpha_bias=np.asarray(alpha_bias, np.float32),
    )
    out, _ = _run(inputs)
    return out


# revision 9
# speedup vs baseline: 1.0994x; 1.0749x over previous
# Self-contained Trainium2 Bass kernel for nn_MultiInputLSTMCell.
#
# Reference computation (all fp32):
#   pre   = h0 @ W_hh + bias + input_ @ W_ih          # (1, 3H)
#   i, o  = sigmoid(pre[:, :H]), sigmoid(pre[:, H:2H])
#   g     = tanh(pre[:, 2H:])
#   awi   = input_ @ aW_ih + a_bias                   # (1, H)
#   awh   = c_input @ aW_hh                           # (C, H)
#   alpha = sigmoid(awi + awh)                        # (C, H)
#   w     = exp([i; alpha]); w /= w.sum(0)            # (C+1, H)
#   c1    = (([g; c_input]) * w).sum(0)               # (1, H)
#   h1    = o * tanh(c1)
#
# Strategy: tensor-parallel over the hidden (output-column) dim across 8
# cores (HS = 256 columns each).  All elementwise/reduction work after the
# matmuls is local to a hidden shard, so no collectives are needed; the
# host scatters weight columns and gathers the (1, 256) h1/c1 shards.
#
# Per-core layout: the C axis lives on SBUF partitions, hidden on the free
# dim.  Weights are streamed through the PE as the *moving* operand in
# float32r (single-pass fp32: 1 col/cycle vs 4 for exact fp32); the tiny
# activation vectors are the stationary lhsT.  The (C+1)-axis softmax-style
# reduction is a K=65 ones-vector matmul kept in exact fp32.  The kernel is
# memory-bound on the ~16.6 MB of weights each core reads; weight DMAs are
# batched into 2-3 MB transfers on the sync HWDGE ring while small operands
# ride the scalar ring.

import numpy as np

import concourse.bass as bass
import concourse.tile as tile
from concourse import bacc, mybir
from concourse.bass_utils import run_bass_kernel_spmd

NCORES = 8
H = 2048          # hidden size
IN = 2048         # input size
C = 64            # number of skip-word cell states
HS = H // NCORES  # hidden shard per core = 256
KG = IN + H       # gates contraction dim = 4096
F32 = mybir.dt.float32
F32R = mybir.dt.float32r

_nc_cache = None


def _build_nc():
    """Build the single-core Bass program (same program runs on all 8 cores)."""
    nc = bacc.Bacc(
        "TRN2",
        target_bir_lowering=False,
        debug=False,
        enable_asserts=False,
        name="multi_input_lstm_cell",
    )

    # DRAM I/O (per-core shards; shapes identical on every core)
    wg = nc.dram_tensor("wg", [KG, 3 * HS], F32R, kind="ExternalInput").ap()
    wai = nc.dram_tensor("wai", [IN, HS], F32R, kind="ExternalInput").ap()
    wah = nc.dram_tensor("wah", [H, HS], F32R, kind="ExternalInput").ap()
    # bab[0, 0:768] = gates bias shard, bab[0, 768:1024] = alpha bias shard
    bab = nc.dram_tensor("bab", [1, 4 * HS], F32, kind="ExternalInput").ap()
    cs = nc.dram_tensor("cs", [C, HS], F32, kind="ExternalInput").ap()
    xt = nc.dram_tensor("xt", [128, KG // 128], F32R, kind="ExternalInput").ap()
    ct = nc.dram_tensor("ct", [H, C], F32R, kind="ExternalInput").ap()
    # hc[0, 0:256] = c1 shard, hc[0, 256:512] = h1 shard (one output DMA)
    hc = nc.dram_tensor("hc", [1, 2 * HS], F32, kind="ExternalOutput").ap()

    with tile.TileContext(nc) as tc:
        _emit(tc, wg, wai, wah, bab, cs, xt, ct, hc)

    nc.compile()
    return nc


def _emit(tc, wg, wai, wah, bab, cs, xt, ct, hc):
    from contextlib import ExitStack

    nc = tc.nc
    KO_G = KG // 128          # 32 contraction chunks for the gates matmul
    KO_A = IN // 128          # 16 contraction chunks for the alpha matmuls
    GSUB = 4                  # gates k-chunks per DMA (tile = [128, 4, 768] = 1.5 MB)
    SIG = mybir.ActivationFunctionType.Sigmoid
    TANH = mybir.ActivationFunctionType.Tanh
    EXP = mybir.ActivationFunctionType.Exp

    with ExitStack() as ctx:
        singles = ctx.enter_context(tc.tile_pool(name="singles", bufs=1))
        wg_pool = ctx.enter_context(tc.tile_pool(name="wg_pool", bufs=4))
        psum = ctx.enter_context(tc.tile_pool(name="psum", bufs=1, space="PSUM"))

        # ---- sync (SP) HWDGE ring: bias pack, xt, then the gates stream.
        # The tiny 1-partition bias DMA goes first so its descriptor spray
        # finishes before the big weight stream occupies the SDMA engines.
        bab_t = singles.tile([1, 4 * HS], F32, tag="bab")
        nc.sync.dma_start(out=bab_t[:], in_=bab)
        b_t = bab_t[:, 0 : 3 * HS]
        ab_t = bab_t[:, 3 * HS : 4 * HS]

        xt_t = singles.tile([128, KO_G], F32R, tag="xt")
        nc.sync.dma_start(out=xt_t[:], in_=xt)

        # ---- scalar (ACT) HWDGE ring: alpha weight stream + merge data --
        wai_t = singles.tile([128, KO_A, HS], F32R, tag="wai")
        nc.scalar.dma_start(out=wai_t[:], in_=wai.rearrange("(ko ki) n -> ki ko n", ki=128))
        ct_t = singles.tile([128, KO_A, C], F32R, tag="ct")
        nc.scalar.dma_start(out=ct_t[:], in_=ct.rearrange("(ko ki) c -> ki ko c", ki=128))
        wah_t = singles.tile([128, KO_A, HS], F32R, tag="wah")
        nc.scalar.dma_start(out=wah_t[:], in_=wah.rearrange("(ko ki) n -> ki ko n", ki=128))

        # merge tile rows: [c_input-shard; g]  (C+1 = 65 partitions; the
        # singleton gate row lives at partition 64 — compute instructions
        # only support start partitions {0, 32, 64})
        mg_t = singles.tile([C + 1, HS], F32, tag="mg")
        nc.scalar.dma_start(out=mg_t[0:C, :], in_=cs)

        ew_t = singles.tile([C + 1, HS], F32, tag="ew")

        ones_r = singles.tile([C + 1, 1], F32, tag="ones_r")   # reduction lhsT
        nc.vector.memset(ones_r[:], 1.0)
        ones_b = singles.tile([1, C], F32, tag="ones_b")       # broadcast/bias lhsT
        nc.vector.memset(ones_b[:], 1.0)

        # Pre-warm the ACT engine's exp table (slot 1) while everything is
        # idle so the mid-kernel exp doesn't pay the ~1.3 µs table load.
        warm_t = singles.tile([1, 1], F32, tag="warm")
        nc.vector.memset(warm_t[:], 0.0)
        nc.scalar.activation(out=warm_t[:], in_=warm_t[:], func=EXP)

        # ---- PSUM tiles ----------------------------------------------
        pg_a = psum.tile([1, 512], F32, tag="pg_a")    # gates cols 0..512 (i, o)
        pg_b = psum.tile([1, HS], F32, tag="pg_b")     # gates cols 512..768 (g)
        pwi = psum.tile([1, HS], F32, tag="pwi")       # alpha_wi row
        pal = psum.tile([C, HS], F32, tag="pal")       # alpha pre-activation
        ps0 = psum.tile([1, HS], F32, tag="ps0")       # sum(exp(logits))
        ps1 = psum.tile([1, HS], F32, tag="ps1")       # sum(merge * exp(logits))

        # PE emission order tracks DMA arrival order so the in-order PE
        # queue never stalls behind late data: bias (arrives ~3 µs),
        # alpha_wi, gates chunk 0, alpha_wh + broadcast, remaining gates.

        # gates bias rows via K=1 rank-1 matmuls (opens both PSUM groups)
        nc.tensor.matmul(pg_a[:], lhsT=ones_b[0:1, 0:1], rhs=b_t[:, 0:512],
                         start=True, stop=False)
        nc.tensor.matmul(pg_b[:], lhsT=ones_b[0:1, 0:1], rhs=b_t[:, 512 : 3 * HS],
                         start=True, stop=False)

        # ---- alpha_wi = input_ @ aW_ih  (input_ = xt cols 16..31) -----
        for ko in range(KO_A):
            nc.tensor.matmul(
                pwi[:],
                lhsT=xt_t[:, KO_A + ko : KO_A + ko + 1],
                rhs=wai_t[:, ko, :],
                start=(ko == 0),
                stop=(ko == KO_A - 1),
            )

        wg_r = wg.rearrange("(ko km ki) n -> ko ki km n", ki=128, km=GSUB)
        wg_tiles = []

        def gates_chunk(ko):
            wg_t = wg_pool.tile([128, GSUB, 3 * HS], F32R, tag="wg")
            nc.sync.dma_start(out=wg_t[:], in_=wg_r[ko])
            for km in range(GSUB):
                kk = ko * GSUB + km
                nc.tensor.matmul(
                    pg_a[:],
                    lhsT=xt_t[:, kk : kk + 1],
                    rhs=wg_t[:, km, 0:512],
                    start=False,
                    stop=(kk == KO_G - 1),
                )
                nc.tensor.matmul(
                    pg_b[:],
                    lhsT=xt_t[:, kk : kk + 1],
                    rhs=wg_t[:, km, 512 : 3 * HS],
                    start=False,
                    stop=(kk == KO_G - 1),
                )

        gates_chunk(0)

        # ---- alpha pre = c_input @ aW_hh  (+ broadcast wi row) --------
        for ko in range(KO_A):
            nc.tensor.matmul(
                pal[:],
                lhsT=ct_t[:, ko, :],
                rhs=wah_t[:, ko, :],
                start=(ko == 0),
                stop=False,
            )
        # wi row (+ alpha_bias) to SBUF, then broadcast-add into pal via a
        # K=1 rank-1 matmul with a ones column.
        wi_t = singles.tile([1, HS], F32, tag="wi")
        nc.vector.tensor_add(out=wi_t[:], in0=pwi[:], in1=ab_t[:])
        nc.tensor.matmul(
            pal[:], lhsT=ones_b[0:1, 0:C], rhs=wi_t[:], start=False, stop=True,
        )
        # alpha rows of the exp-normalize input: ew[0:64] = sigmoid(pal)
        # (runs mid-kernel on the otherwise idle ACT engine)
        nc.scalar.activation(out=ew_t[0:C, :], in_=pal[:], func=SIG)

        for ko in range(1, KO_G // GSUB):
            gates_chunk(ko)

        # ---- tail: activations, exp-normalize, h1/c1 ------------------
        # ew row 64 = i gate; mg row 64 = g candidate
        nc.scalar.activation(out=ew_t[C : C + 1, :], in_=pg_a[:, 0:HS], func=SIG)
        nc.scalar.activation(out=mg_t[C : C + 1, :], in_=pg_b[:], func=TANH)
        # exp of the [alpha; i] logits (rows 0..64)
        nc.scalar.activation(out=ew_t[:], in_=ew_t[:], func=EXP)
        # o gate is only needed for the final product — keep it off the
        # critical path (after exp on the ACT queue).
        og_t = singles.tile([1, HS], F32, tag="og")
        nc.scalar.activation(out=og_t[:], in_=pg_a[:, HS:512], func=SIG)
        # merge * exp(logits)
        nc.vector.tensor_mul(out=mg_t[:], in0=mg_t[:], in1=ew_t[:])

        # ---- reduce over the (C+1) axis via ones-matmul (K = 65) ------
        nc.tensor.matmul(ps0[:], lhsT=ones_r[:], rhs=ew_t[:], start=True, stop=True)
        nc.tensor.matmul(ps1[:], lhsT=ones_r[:], rhs=mg_t[:], start=True, stop=True)

        # ---- c1 = ps1 / ps0 ; h1 = o * tanh(c1) -----------------------
        # s0 = sum of 65 exp values in [1, e] — safely normal, so the
        # fast reciprocal approximation (~18 good bits) is plenty.
        r_t = singles.tile([1, HS], F32, tag="r")
        nc.vector.reciprocal_approx_fast(out=r_t[:], in_=ps0[:])
        hc_t = singles.tile([1, 2 * HS], F32, tag="hc")
        c1_t = hc_t[:, 0:HS]
        nc.vector.tensor_mul(out=c1_t, in0=ps1[:], in1=r_t[:])

        t_t = singles.tile([1, HS], F32, tag="t")
        nc.scalar.activation(out=t_t[:], in_=c1_t, func=TANH)
        nc.vector.tensor_mul(out=hc_t[:, HS : 2 * HS], in0=og_t[:], in1=t_t[:])

        nc.sync.dma_start(out=hc, in_=hc_t[:])

def _shard_inputs(input_, c_input, h0, c0, weight_ih, weight_hh,
                  alpha_weight_ih, alpha_weight_hh, bias, alpha_bias):
    """Host-side scatter: column-shard the weights over the hidden dim."""
    f32 = np.float32
    x_comb = np.concatenate([h0[0], input_[0]]).astype(f32)          # (4096,)
    xt = np.ascontiguousarray(x_comb.reshape(KG // 128, 128).T)      # (128, 32)
    ct = np.ascontiguousarray(c_input.T.astype(f32))                 # (2048, 64)

    in_maps = []
    for k in range(NCORES):
        cols = np.s_[k * HS : (k + 1) * HS]
        gcols = np.r_[0 * H + k * HS : 0 * H + (k + 1) * HS,
                      1 * H + k * HS : 1 * H + (k + 1) * HS,
                      2 * H + k * HS : 2 * H + (k + 1) * HS]
        wg = np.ascontiguousarray(
            np.concatenate([weight_hh[:, gcols], weight_ih[:, gcols]], axis=0)
        ).astype(f32)                                                # (4096, 768)
        in_maps.append({
            "wg": wg,
            "wai": np.ascontiguousarray(alpha_weight_ih[:, cols]).astype(f32),
            "wah": np.ascontiguousarray(alpha_weight_hh[:, cols]).astype(f32),
            "bab": np.concatenate(
                [bias[gcols], alpha_bias[cols]])[None, :].astype(f32),
            "cs": np.ascontiguousarray(c_input[:, cols]).astype(f32),
            "xt": xt,
            "ct": ct,
        })
    return in_maps


def _run(inputs, trace=False):
    global _nc_cache
    if _nc_cache is None:
        _nc_cache = _build_nc()
    nc = _nc_cache
    in_maps = _shard_inputs(**inputs)
    res = run_bass_kernel_spmd(nc, in_maps, core_ids=list(range(NCORES)), trace=trace)
    h1 = np.concatenate(
        [res.results[k]["hc"][:, HS : 2 * HS] for k in range(NCORES)], axis=1)
    c1 = np.concatenate(
        [res.results[k]["hc"][:, 0:HS] for k in range(NCORES)], axis=1)
    return (h1.astype(np.float32), c1.astype(np.float32)), res


def kernel(input_, c_input, h0, c0, weight_ih, weight_hh,
           alpha_weight_ih, alpha_weight_hh, bias, alpha_bias):
    inputs = dict(
        input_=np.asarray(input_, np.float32),
        c_input=np.asarray(c_input, np.float32),
        h0=np.asarray(h0, np.float32),
        c0=np.asarray(c0, np.float32),
        weight_ih=np.asarray(weight_ih, np.float32),
        weight_hh=np.asarray(weight_hh, np.float32),
        alpha_weight_ih=np.asarray(alpha_weight_ih, np.float32),
        alpha_weight_hh=np.asarray(alpha_weight_hh, np.float32),
        bias=np.asarray(bias, np.float32),
        alpha_bias=np.asarray(alpha_bias, np.float32),
    )
    out, _ = _run(inputs)
    return out


# revision 10
# speedup vs baseline: 1.3109x; 1.1924x over previous
# Self-contained Trainium2 Bass kernel for nn_MultiInputLSTMCell.
#
# Reference computation (all fp32):
#   pre   = h0 @ W_hh + bias + input_ @ W_ih          # (1, 3H)
#   i, o  = sigmoid(pre[:, :H]), sigmoid(pre[:, H:2H])
#   g     = tanh(pre[:, 2H:])
#   awi   = input_ @ aW_ih + a_bias                   # (1, H)
#   awh   = c_input @ aW_hh                           # (C, H)
#   alpha = sigmoid(awi + awh)                        # (C, H)
#   w     = exp([i; alpha]); w /= w.sum(0)            # (C+1, H)
#   c1    = (([g; c_input]) * w).sum(0)               # (1, H)
#   h1    = o * tanh(c1)
#
# Strategy: tensor-parallel over the hidden (output-column) dim across 8
# cores (HS = 256 columns each).  All elementwise/reduction work after the
# matmuls is local to a hidden shard, so no collectives are needed; the
# host scatters weight columns and gathers the (1, 256) h1/c1 shards.
#
# Per-core layout: the C axis lives on SBUF partitions, hidden on the free
# dim.  Weights are streamed through the PE as the *moving* operand in
# float32r (single-pass fp32: 1 col/cycle vs 4 for exact fp32); the tiny
# activation vectors are the stationary lhsT.  The (C+1)-axis softmax-style
# reduction is a K=65 ones-vector matmul kept in exact fp32.  The kernel is
# memory-bound on the ~16.6 MB of weights each core reads; weight DMAs are
# batched into 2-3 MB transfers on the sync HWDGE ring while small operands
# ride the scalar ring.

import numpy as np

import concourse.bass as bass
import concourse.tile as tile
from concourse import bacc, mybir
from concourse.bass_utils import run_bass_kernel_spmd

NCORES = 8
H = 2048          # hidden size
IN = 2048         # input size
C = 64            # number of skip-word cell states
HS = H // NCORES  # hidden shard per core = 256
KG = IN + H       # gates contraction dim = 4096
F32 = mybir.dt.float32
F32R = mybir.dt.float32r

_nc_cache = None


def _build_nc():
    """Build the single-core Bass program (same program runs on all 8 cores)."""
    nc = bacc.Bacc(
        "TRN2",
        target_bir_lowering=False,
        debug=False,
        enable_asserts=False,
        name="multi_input_lstm_cell",
    )

    # DRAM I/O (per-core shards; shapes identical on every core)
    wg = nc.dram_tensor("wg", [KG, 3 * HS], F32R, kind="ExternalInput").ap()
    wai = nc.dram_tensor("wai", [IN, HS], F32R, kind="ExternalInput").ap()
    wah = nc.dram_tensor("wah", [H, HS], F32R, kind="ExternalInput").ap()
    # bab[0, 0:768] = gates bias shard, bab[0, 768:1024] = alpha bias shard
    bab = nc.dram_tensor("bab", [1, 4 * HS], F32, kind="ExternalInput").ap()
    cs = nc.dram_tensor("cs", [C, HS], F32, kind="ExternalInput").ap()
    xt = nc.dram_tensor("xt", [128, KG // 128], F32R, kind="ExternalInput").ap()
    ct = nc.dram_tensor("ct", [H, C], F32R, kind="ExternalInput").ap()
    # hc[0, 0:256] = c1 shard, hc[0, 256:512] = h1 shard (one output DMA)
    hc = nc.dram_tensor("hc", [1, 2 * HS], F32, kind="ExternalOutput").ap()

    with tile.TileContext(nc) as tc:
        _emit(tc, wg, wai, wah, bab, cs, xt, ct, hc)

    nc.compile()
    return nc


def _emit(tc, wg, wai, wah, bab, cs, xt, ct, hc):
    from contextlib import ExitStack

    nc = tc.nc
    KO_G = KG // 128          # 32 contraction chunks for the gates matmul
    KO_A = IN // 128          # 16 contraction chunks for the alpha matmuls
    GSUB = 4                  # gates k-chunks per DMA (tile = [128, 4, 768] = 1.5 MB)
    SIG = mybir.ActivationFunctionType.Sigmoid
    TANH = mybir.ActivationFunctionType.Tanh
    EXP = mybir.ActivationFunctionType.Exp

    with ExitStack() as ctx:
        singles = ctx.enter_context(tc.tile_pool(name="singles", bufs=1))
        wg_pool = ctx.enter_context(tc.tile_pool(name="wg_pool", bufs=6))
        psum = ctx.enter_context(tc.tile_pool(name="psum", bufs=1, space="PSUM"))

        # ---- single big-transfer stream on the sync (SP) HWDGE ring, in
        # exact PE consumption order (the scalar ring moves large tensors
        # ~3x slower, so only tiny late-consumed loads go there).  The tiny
        # 1-partition bias spray goes first, before the weight stream
        # occupies the SDMA engines.
        bab_t = singles.tile([1, 4 * HS], F32, tag="bab")
        nc.sync.dma_start(out=bab_t[:], in_=bab)
        b_t = bab_t[:, 0 : 3 * HS]
        ab_t = bab_t[:, 3 * HS : 4 * HS]

        xt_t = singles.tile([128, KO_G], F32R, tag="xt")
        nc.sync.dma_start(out=xt_t[:], in_=xt)

        wai_t = singles.tile([128, KO_A, HS], F32R, tag="wai")
        nc.sync.dma_start(out=wai_t[:], in_=wai.rearrange("(ko ki) n -> ki ko n", ki=128))

        # ct / wah issued later, between the first gates chunks (see below)
        ct_t = singles.tile([128, KO_A, C], F32R, tag="ct")
        wah_t = singles.tile([128, KO_A, HS], F32R, tag="wah")

        # merge tile rows: [c_input-shard; g]  (C+1 = 65 partitions; the
        # singleton gate row lives at partition 64 — compute instructions
        # only support start partitions {0, 32, 64})
        mg_t = singles.tile([C + 1, HS], F32, tag="mg")
        nc.scalar.dma_start(out=mg_t[0:C, :], in_=cs)

        ew_t = singles.tile([C + 1, HS], F32, tag="ew")

        ones_r = singles.tile([C + 1, 1], F32, tag="ones_r")   # reduction lhsT
        nc.vector.memset(ones_r[:], 1.0)
        ones_b = singles.tile([1, C], F32, tag="ones_b")       # broadcast/bias lhsT
        nc.vector.memset(ones_b[:], 1.0)

        # Pre-warm the ACT engine's exp table (slot 1) while everything is
        # idle so the mid-kernel exp doesn't pay the ~1.3 µs table load.
        warm_t = singles.tile([1, 1], F32, tag="warm")
        nc.vector.memset(warm_t[:], 0.0)
        nc.scalar.activation(out=warm_t[:], in_=warm_t[:], func=EXP)

        # ---- PSUM tiles ----------------------------------------------
        pg_a = psum.tile([1, 512], F32, tag="pg_a")    # gates cols 0..512 (i, o)
        pg_b = psum.tile([1, HS], F32, tag="pg_b")     # gates cols 512..768 (g)
        pwi = psum.tile([1, HS], F32, tag="pwi")       # alpha_wi row
        pal = psum.tile([C, HS], F32, tag="pal")       # alpha pre-activation
        ps0 = psum.tile([1, HS], F32, tag="ps0")       # sum(exp(logits))
        ps1 = psum.tile([1, HS], F32, tag="ps1")       # sum(merge * exp(logits))

        # PE emission order tracks DMA arrival order so the in-order PE
        # queue never stalls behind late data: bias (arrives ~3 µs),
        # alpha_wi, gates chunk 0, alpha_wh + broadcast, remaining gates.

        # gates bias rows via K=1 rank-1 matmuls (opens both PSUM groups)
        nc.tensor.matmul(pg_a[:], lhsT=ones_b[0:1, 0:1], rhs=b_t[:, 0:512],
                         start=True, stop=False)
        nc.tensor.matmul(pg_b[:], lhsT=ones_b[0:1, 0:1], rhs=b_t[:, 512 : 3 * HS],
                         start=True, stop=False)

        # ---- alpha_wi = input_ @ aW_ih  (input_ = xt cols 16..31) -----
        for ko in range(KO_A):
            nc.tensor.matmul(
                pwi[:],
                lhsT=xt_t[:, KO_A + ko : KO_A + ko + 1],
                rhs=wai_t[:, ko, :],
                start=(ko == 0),
                stop=(ko == KO_A - 1),
            )

        wg_r = wg.rearrange("(ko km ki) n -> ko ki km n", ki=128, km=GSUB)
        wg_tiles = []

        def gates_chunk(ko):
            wg_t = wg_pool.tile([128, GSUB, 3 * HS], F32R, tag="wg")
            nc.sync.dma_start(out=wg_t[:], in_=wg_r[ko])
            for km in range(GSUB):
                kk = ko * GSUB + km
                nc.tensor.matmul(
                    pg_a[:],
                    lhsT=xt_t[:, kk : kk + 1],
                    rhs=wg_t[:, km, 0:512],
                    start=False,
                    stop=(kk == KO_G - 1),
                )
                nc.tensor.matmul(
                    pg_b[:],
                    lhsT=xt_t[:, kk : kk + 1],
                    rhs=wg_t[:, km, 512 : 3 * HS],
                    start=False,
                    stop=(kk == KO_G - 1),
                )

        gates_chunk(0)
        gates_chunk(1)

        # ---- alpha pre = c_input @ aW_hh  (+ broadcast wi row) --------
        nc.sync.dma_start(out=ct_t[:], in_=ct.rearrange("(ko ki) c -> ki ko c", ki=128))
        nc.sync.dma_start(out=wah_t[:], in_=wah.rearrange("(ko ki) n -> ki ko n", ki=128))
        for ko in range(KO_A):
            nc.tensor.matmul(
                pal[:],
                lhsT=ct_t[:, ko, :],
                rhs=wah_t[:, ko, :],
                start=(ko == 0),
                stop=False,
            )
        # wi row (+ alpha_bias) to SBUF, then broadcast-add into pal via a
        # K=1 rank-1 matmul with a ones column.
        wi_t = singles.tile([1, HS], F32, tag="wi")
        nc.vector.tensor_add(out=wi_t[:], in0=pwi[:], in1=ab_t[:])
        nc.tensor.matmul(
            pal[:], lhsT=ones_b[0:1, 0:C], rhs=wi_t[:], start=False, stop=True,
        )
        # alpha rows of the exp-normalize input: ew[0:64] = sigmoid(pal)
        # (runs mid-kernel on the otherwise idle ACT engine)
        nc.scalar.activation(out=ew_t[0:C, :], in_=pal[:], func=SIG)

        for ko in range(2, KO_G // GSUB):
            gates_chunk(ko)

        # ---- tail: activations, exp-normalize, h1/c1 ------------------
        # ew row 64 = i gate; mg row 64 = g candidate
        nc.scalar.activation(out=ew_t[C : C + 1, :], in_=pg_a[:, 0:HS], func=SIG)
        nc.scalar.activation(out=mg_t[C : C + 1, :], in_=pg_b[:], func=TANH)
        # exp of the [alpha; i] logits (rows 0..64)
        nc.scalar.activation(out=ew_t[:], in_=ew_t[:], func=EXP)
        # o gate is only needed for the final product — keep it off the
        # critical path (after exp on the ACT queue).
        og_t = singles.tile([1, HS], F32, tag="og")
        nc.scalar.activation(out=og_t[:], in_=pg_a[:, HS:512], func=SIG)
        # merge * exp(logits)
        nc.vector.tensor_mul(out=mg_t[:], in0=mg_t[:], in1=ew_t[:])

        # ---- reduce over the (C+1) axis via ones-matmul (K = 65) ------
        nc.tensor.matmul(ps0[:], lhsT=ones_r[:], rhs=ew_t[:], start=True, stop=True)
        nc.tensor.matmul(ps1[:], lhsT=ones_r[:], rhs=mg_t[:], start=True, stop=True)

        # ---- c1 = ps1 / ps0 ; h1 = o * tanh(c1) -----------------------
        # s0 = sum of 65 exp values in [1, e] — safely normal, so the
        # fast reciprocal approximation (~18 good bits) is plenty.
        r_t = singles.tile([1, HS], F32, tag="r")
        nc.vector.reciprocal_approx_fast(out=r_t[:], in_=ps0[:])
        hc_t = singles.tile([1, 2 * HS], F32, tag="hc")
        c1_t = hc_t[:, 0:HS]
        nc.vector.tensor_mul(out=c1_t, in0=ps1[:], in1=r_t[:])

        t_t = singles.tile([1, HS], F32, tag="t")
        nc.scalar.activation(out=t_t[:], in_=c1_t, func=TANH)
        nc.vector.tensor_mul(out=hc_t[:, HS : 2 * HS], in0=og_t[:], in1=t_t[:])

        nc.sync.dma_start(out=hc, in_=hc_t[:])

def _shard_inputs(input_, c_input, h0, c0, weight_ih, weight_hh,
                  alpha_weight_ih, alpha_weight_hh, bias, alpha_bias):
    """Host-side scatter: column-shard the weights over the hidden dim."""
    f32 = np.float32
    x_comb = np.concatenate([h0[0], input_[0]]).astype(f32)          # (4096,)
    xt = np.ascontiguousarray(x_comb.reshape(KG // 128, 128).T)      # (128, 32)
    ct = np.ascontiguousarray(c_input.T.astype(f32))                 # (2048, 64)

    in_maps = []
    for k in range(NCORES):
        cols = np.s_[k * HS : (k + 1) * HS]
        gcols = np.r_[0 * H + k * HS : 0 * H + (k + 1) * HS,
                      1 * H + k * HS : 1 * H + (k + 1) * HS,
                      2 * H + k * HS : 2 * H + (k + 1) * HS]
        wg = np.ascontiguousarray(
            np.concatenate([weight_hh[:, gcols], weight_ih[:, gcols]], axis=0)
        ).astype(f32)                                                # (4096, 768)
        in_maps.append({
            "wg": wg,
            "wai": np.ascontiguousarray(alpha_weight_ih[:, cols]).astype(f32),
            "wah": np.ascontiguousarray(alpha_weight_hh[:, cols]).astype(f32),
            "bab": np.concatenate(
                [bias[gcols], alpha_bias[cols]])[None, :].astype(f32),
            "cs": np.ascontiguousarray(c_input[:, cols]).astype(f32),
            "xt": xt,
            "ct": ct,
        })
    return in_maps


def _run(inputs, trace=False):
    global _nc_cache
    if _nc_cache is None:
        _nc_cache = _build_nc()
    nc = _nc_cache
    in_maps = _shard_inputs(**inputs)
    res = run_bass_kernel_spmd(nc, in_maps, core_ids=list(range(NCORES)), trace=trace)
    h1 = np.concatenate(
        [res.results[k]["hc"][:, HS : 2 * HS] for k in range(NCORES)], axis=1)
    c1 = np.concatenate(
        [res.results[k]["hc"][:, 0:HS] for k in range(NCORES)], axis=1)
    return (h1.astype(np.float32), c1.astype(np.float32)), res


def kernel(input_, c_input, h0, c0, weight_ih, weight_hh,
           alpha_weight_ih, alpha_weight_hh, bias, alpha_bias):
    inputs = dict(
        input_=np.asarray(input_, np.float32),
        c_input=np.asarray(c_input, np.float32),
        h0=np.asarray(h0, np.float32),
        c0=np.asarray(c0, np.float32),
        weight_ih=np.asarray(weight_ih, np.float32),
        weight_hh=np.asarray(weight_hh, np.float32),
        alpha_weight_ih=np.asarray(alpha_weight_ih, np.float32),
        alpha_weight_hh=np.asarray(alpha_weight_hh, np.float32),
        bias=np.asarray(bias, np.float32),
        alpha_bias=np.asarray(alpha_bias, np.float32),
    )
    out, _ = _run(inputs)
    return out
